# revision 50
# baseline (speedup 1.0000x reference)
"""Dual attention (DANet-style spatial + channel attention) on 8 Trainium2
NeuronCores.

Sharding: data-parallel over batch B=4, and each batch's output positions
(m in [0, 4096)) split in half across 2 cores -> 8 identical single-core
programs, no collectives. Each core receives its batch's full x (columns
rotated so the core's own m-half comes first -- attention sums over key
positions n are order-invariant) and produces out[:, m_slice].

Per-core math (x: [512, 4096], m-chunk: 2048 positions):
  spatial:  q=Wq@xq+bq; k=Wk@x+bk; E^T[n,m]=k[:,n].q[:,m]; P=exp(E^T) bf16
            Zb[*,m]=sum_n P[n,m]/cp  (PE matmul, all-(1/cp) lhsT -> already
            broadcast across partitions); P8=e4m3(P * cp/Z) -- softmax
            weights scaled to <=cp, exactly representable in TRN fp8 e4m3
            (max 240); U[c,m]=sum_n v8[n,c]P8[n,m] in DoubleRow fp8 (2
            key-tiles per PE instruction, 2x rate);
            out_s = (gamma_s/(cp*cv))*U + x[:,m]
  channel:  p=Wd@x+bd (full N); pT tiles via PE transpose;
            e=pT^T@pT; c_attn=softmax(rowmax(e)-e);
            c2=gamma_c*(c_attn@p)[:,m]+p[:,m]; channel = Wu@c2+bu
  out = out_s + channel

Performance structure (evolved from a 272us bf16 baseline; the PE moving
stream paces every free-512 matmul at ~380ns regardless of dtype, so the
currency is matmul-instruction count, and gaps are doubly expensive due to
p-state ramping):
  - attn@V and the Wv projection in fp8 e4m3 DoubleRow (2 k-tiles per
    instruction); softmax weights P/Z <= 1 always fit fp8 (4.4e-3
    scale-rel absmax err vs the 2e-2 budget, validated offline).
  - Z accumulated on the PE (broadcast ones-matmul), DVE only normalizes
    P into fp8 and does epilogue combines.
  - pT built by PE transposes of the full-N p projection (replaces 160
    small matmuls with 32 free-512 ones + 32 cheap transposes); p's
    m-half doubles as pc.
  - chunk-0 E/exp/Zb braided into the serial channel-attention chain;
    chunk mc+1's E/exp/Zb braided into chunk mc's fp8 U matmuls, so the
    PE never drains.
"""
import sys

sys.path.insert(0, '/opt/trn_rl_repo')

import ml_dtypes
import numpy as np

import concourse.bass as bass
import concourse.tile as tile
from concourse import bacc, bass_utils, mybir
from concourse.masks import make_identity

# Problem shapes (fixed by the task spec)
B, C, WIDTH, HEIGHT = 4, 512, 64, 64
N = WIDTH * HEIGHT      # 4096 spatial positions
DK = 64                 # attention inner dim (and channel-attn dim)
NCORES = 8
M = N // 2              # 2048 output positions per core
P = 128
KC = C // P             # 4 input-channel chunks
NT = N // P             # 32 key-position tiles
FREE = 512              # matmul moving free dim (one PSUM bank of fp32)
MCH = M // FREE         # 4 m-chunks per core
CCH = C // P            # 4 output-channel chunks
CP = 128.0              # fp8 softmax-weight scale (weights <= CP <= 240)
CV = 24.0               # fp8 value scale (|cv*v| <= ~125 < 240)

F32 = mybir.dt.float32
F16 = mybir.dt.float16
BF16 = mybir.dt.bfloat16
F8 = mybir.dt.float8e4
AX = mybir.AxisListType
ALU = mybir.AluOpType
ACTF = mybir.ActivationFunctionType
DR = mybir.MatmulPerfMode.DoubleRow

# byte layout of the packed-constants image (per partition)
OFF_WQ, OFF_WD, OFF_WK = 0, 512, 1024
OFF_BQ, OFF_BK, OFF_BD = 1536, 1540, 1544
OFF_BU = 1548            # [128, 4] f32
OFF_GC = 1564
OFF_GSP = 1568           # [128, 1] f32: gamma_s / (CP*CV)
PKB = 1572


def _build_program(tc, io):
    nc = tc.nc
    x_d, x8_d, out_d = io['x'], io['x8'], io['out']

    const_cm = tc.tile_pool(name='const', bufs=1)
    const = const_cm.__enter__()

    # ---- persistent SBUF tensors ----
    pk_sb = const.tile([P, PKB], mybir.dt.uint8)
    nc.sync.dma_start(pk_sb[:], io['consts'][:])
    wq_sb = pk_sb[:, OFF_WQ:OFF_WQ + 512].bitcast(F16).rearrange(
        "p (kc d) -> p kc d", kc=KC)
    wd_sb = pk_sb[:, OFF_WD:OFF_WD + 512].bitcast(F16).rearrange(
        "p (kc d) -> p kc d", kc=KC)
    wk_sb = pk_sb[:, OFF_WK:OFF_WK + 512].bitcast(F16).rearrange(
        "p (kc d) -> p kc d", kc=KC)
    bq_sb = pk_sb[0:DK, OFF_BQ:OFF_BQ + 4].bitcast(F32)
    bk_sb = pk_sb[0:DK, OFF_BK:OFF_BK + 4].bitcast(F32)
    bd_sb = pk_sb[0:DK, OFF_BD:OFF_BD + 4].bitcast(F32)
    bu_sb = pk_sb[:, OFF_BU:OFF_BU + 16].bitcast(F32)
    gc_sb = pk_sb[0:DK, OFF_GC:OFF_GC + 4].bitcast(F32)
    gsp_sb = pk_sb[:, OFF_GSP:OFF_GSP + 4].bitcast(F32)
    bvrow_sb = const.tile([1, C], F32)     # CV*bv row and Wu^T: own DMAs,
    wu_sb = const.tile([DK, C], F16)       # off the consts critical path

    wv8_sb = const.tile([P, KC, C], F8)

    ones_zb = const.tile([P, P], BF16)     # lhsT for Z broadcast-sum (=1/CP)
    nc.vector.memset(ones_zb[:], 1.0 / CP)
    ident16 = const.tile([DK, DK], F16)    # for PE transposes
    make_identity(nc, ident16[:])

    k_sb = const.tile([DK, N], F16)        # keys,    [d, n]
    q_sb = const.tile([DK, M], F16)        # queries, [d, m]
    p_sb_f = const.tile([DK, N], F16)      # channel proj, full N (m-half=pc)
    pT_sb = const.tile([P, NT, DK], F16)   # channel proj transposed
    vT8_sb = const.tile([P, NT, C], F8)    # values transposed fp8, [n, nt, c]
    c2_sb = const.tile([DK, M], F16)       # gamma_c * c_attn@p + p on m-slice
    bvb_sb = const.tile([P, C], F32)       # CV*bv broadcast to all 128 parts
    r_sb = const.tile([P, CCH, M], F32)    # channel-out + x residual

    # ---- DMA: x m-half first, then fp8 m-half, then the rest ----
    xp16_cm = tc.tile_pool(name='xp16', bufs=1)
    xp16 = xp16_cm.__enter__()
    xp8_cm = tc.tile_pool(name='xp8', bufs=1)
    xp8 = xp8_cm.__enter__()
    x_sb = xp16.tile([P, KC, N], F16)
    x8_sb = xp8.tile([P, KC, N], F8)
    # chunk-major dram layout: one contiguous 4KB/2KB run per partition
    for nq in range(4):
        qsl = slice(nq * FREE, (nq + 1) * FREE)
        nc.sync.dma_start(x_sb[:, :, qsl], x_d[:, nq])
    nc.sync.dma_start(wv8_sb[:],
                      io['wv8'].rearrange("(kc p) c -> p kc c", p=P))
    nc.sync.dma_start(bvrow_sb[:], io['bvr'][:])
    nc.sync.dma_start(wu_sb[:], io['wuT'][:])
    for nq in range(4):
        qsl = slice(nq * FREE, (nq + 1) * FREE)
        nc.sync.dma_start(x8_sb[:, :, qsl], x8_d[:, nq])
    for nq in range(4, 8):
        qsl = slice(nq * FREE, (nq + 1) * FREE)
        nc.sync.dma_start(x_sb[:, :, qsl], x_d[:, nq])
    for nq in range(4, 8):
        qsl = slice(nq * FREE, (nq + 1) * FREE)
        nc.sync.dma_start(x8_sb[:, :, qsl], x8_d[:, nq])

    # ---- q projection from the m-half ----
    with tc.tile_pool(name='ps0', bufs=2, space='PSUM') as ps0:
        for j in range(M // FREE):
            sl = slice(j * FREE, (j + 1) * FREE)
            pq = ps0.tile([DK, FREE], F32, tag='pq')
            for kc in range(KC):
                nc.tensor.matmul(pq[:], lhsT=wq_sb[:, kc],
                                 rhs=x_sb[:, kc, sl],
                                 start=(kc == 0), stop=(kc == KC - 1))
            nc.scalar.activation(q_sb[:, sl], pq[:], ACTF.Identity,
                                 bias=bq_sb[:])

    # CV*bv broadcast to [128, C] once
    nc.gpsimd.partition_broadcast(bvb_sb[:], bvrow_sb[:], channels=P)

    ph1_cm = tc.tile_pool(name='ps1', bufs=2, space='PSUM')
    ps1 = ph1_cm.__enter__()
    ph1s_cm = tc.tile_pool(name='ps1s', bufs=2, space='PSUM')
    ps1s = ph1s_cm.__enter__()
    ph1e_cm = tc.tile_pool(name='ps1e', bufs=1, space='PSUM')
    ps1e = ph1e_cm.__enter__()

    def proj_kp(j):
        # k and p (channel proj) over n-window j
        sl = slice(j * FREE, (j + 1) * FREE)
        pk = ps1.tile([DK, FREE], F32, tag='pk')
        for kc in range(KC):
            nc.tensor.matmul(pk[:], lhsT=wk_sb[:, kc], rhs=x_sb[:, kc, sl],
                             start=(kc == 0), stop=(kc == KC - 1))
        nc.scalar.activation(k_sb[:, sl], pk[:], ACTF.Identity, bias=bk_sb[:])
        pp = ps1.tile([DK, FREE], F32, tag='pk')
        for kc in range(KC):
            nc.tensor.matmul(pp[:], lhsT=wd_sb[:, kc], rhs=x_sb[:, kc, sl],
                             start=(kc == 0), stop=(kc == KC - 1))
        nc.scalar.activation(p_sb_f[:, sl], pp[:], ACTF.Identity,
                             bias=bd_sb[:])

    # e accumulated over pT PAIRS: the [128,128] output's two diagonal
    # blocks hold both tiles' Gram terms (cross blocks are ignored) --
    # halves the e-matmul count
    e_ps = ps1e.tile([P, P], F32, tag='e')

    def emit_e(j):
        pair = pT_sb[:, 2 * j:2 * j + 2, :]
        nc.tensor.matmul(e_ps[:], lhsT=pair, rhs=pair,
                         start=(j == 0), stop=(j == NT // 2 - 1))

    def vt8_pt(nt):
        # one vT8 tile (DoubleRow fp8), one pT transpose, one lagged e-accum
        nsl = slice(nt * P, (nt + 1) * P)
        pv = ps1.tile([P, C], F32, tag='pv')
        for i in range(KC // 2):
            nc.tensor.matmul(pv[:], lhsT=x8_sb[:, 2 * i:2 * i + 2, nsl],
                             rhs=wv8_sb[:, 2 * i:2 * i + 2, :],
                             start=(i == 0), stop=(i == KC // 2 - 1),
                             perf_mode=DR)
        nc.vector.tensor_add(vT8_sb[:, nt], in0=pv[:], in1=bvb_sb[:])
        tp = ps1s.tile([P, DK], F16, tag='tp')
        nc.tensor.transpose(tp[:], p_sb_f[:, nsl], ident16[:])
        nc.vector.tensor_copy(pT_sb[:, nt], tp[:])
        if nt >= 3 and nt % 2 == 1:     # pair (nt-3)//2 copied >=2 slots ago
            emit_e((nt - 3) // 2)

    # follow the DMA arrival order: m-half work, then the rest
    for j in range(4):
        proj_kp(j)
    for nt in range(16):
        vt8_pt(nt)
    for j in range(4, 8):
        proj_kp(j)
    for nt in range(16, NT):
        vt8_pt(nt)
    emit_e(NT // 2 - 1)                 # e-accum tail

    # channel-softmax small ops (tiles in const: phase-1 PSUM closes next)
    e_sb = const.tile([DK, DK], F32)
    nc.vector.tensor_copy(e_sb[:], e_ps[0:DK, 0:DK])
    nc.vector.tensor_add(e_sb[:], in0=e_sb[:], in1=e_ps[DK:P, DK:P])
    mn_sb = const.tile([DK, 1], F32)
    nc.vector.tensor_reduce(mn_sb[:], e_sb[:], axis=AX.X, op=ALU.min)
    h_sb = const.tile([DK, DK], F32)
    nc.scalar.activation(h_sb[:], e_sb[:], ACTF.Exp,
                         bias=mn_sb[:], scale=-1.0)
    zc_sb = const.tile([DK, 1], F32)
    nc.vector.tensor_reduce(zc_sb[:], h_sb[:], axis=AX.X, op=ALU.add)
    nc.vector.reciprocal(zc_sb[:], zc_sb[:])
    cat16_sb = const.tile([DK, DK], F16)
    nc.vector.tensor_scalar_mul(cat16_sb[:], in0=h_sb[:], scalar1=zc_sb[:])
    catT_sb = const.tile([DK, DK], F16)

    ph1e_cm.__exit__(None, None, None)
    ph1s_cm.__exit__(None, None, None)
    ph1_cm.__exit__(None, None, None)
    xp8_cm.__exit__(None, None, None)   # x8 dead: free 16KB before p-pool

    # ---- main-loop pools (opened early so chunk-0 E can braid below) ----
    ep_cm = tc.tile_pool(name='epool', bufs=3, space='PSUM')
    epool = ep_cm.__enter__()
    zb_cm = tc.tile_pool(name='zbpool', bufs=1, space='PSUM')
    zbpool = zb_cm.__enter__()
    pp_cm = tc.tile_pool(name='pp', bufs=2)
    ppool = pp_cm.__enter__()
    p8_cm = tc.tile_pool(name='p8p', bufs=5)
    p8pool = p8_cm.__enter__()
    ss_cm = tc.tile_pool(name='ssb', bufs=2)
    ssb = ss_cm.__enter__()
    ot_cm = tc.tile_pool(name='ot', bufs=3)
    otp = ot_cm.__enter__()
    s2_cm = tc.tile_pool(name='s2p', bufs=3)
    s2pool = s2_cm.__enter__()
    s4_cm = tc.tile_pool(name='s4p', bufs=3)
    s4pool = s4_cm.__enter__()
    br1_cm = tc.tile_pool(name='br1', bufs=2, space='PSUM')
    br1 = br1_cm.__enter__()
    br2_cm = tc.tile_pool(name='br2', bufs=1, space='PSUM')
    br2 = br2_cm.__enter__()
    sb2_cm = tc.tile_pool(name='sb2', bufs=2)
    sb2 = sb2_cm.__enter__()

    p_bufs = [None] * MCH
    zb_ps = [None] * MCH
    s2_t = {}
    s4_t = {}
    zstate = {}

    def start_chunk(mc):
        p_bufs[mc] = ppool.tile([P, NT, FREE], BF16, tag='p', name=f'p{mc}')
        zb_ps[mc] = zbpool.tile([P, FREE], F32, tag='zb', name=f'zb{mc}')
        zstate[mc] = [0, 0, 0]

    def emit_E(mc, nt):
        msl = slice(mc * FREE, (mc + 1) * FREE)
        nsl = slice(nt * P, (nt + 1) * P)
        e_t = epool.tile([P, FREE], F32, tag='et')
        nc.tensor.matmul(e_t[:], lhsT=k_sb[:, nsl], rhs=q_sb[:, msl],
                         start=True, stop=True)
        nc.scalar.activation(p_bufs[mc][:, nt, :], e_t[:], ACTF.Exp)

    def emit_s2(mc, j):
        # bf16 pair-sum on DVE (all-2-byte operands hit the 2x path)
        s2 = s2pool.tile([P, FREE], BF16, tag='s2')
        nc.vector.tensor_add(s2[:], in0=p_bufs[mc][:, 2 * j, :],
                             in1=p_bufs[mc][:, 2 * j + 1, :])
        s2_t[(mc, j)] = s2

    def emit_s4(mc, q):
        s4 = s4pool.tile([P, FREE], BF16, tag='s4')
        nc.vector.tensor_add(s4[:], in0=s2_t.pop((mc, 2 * q))[:],
                             in1=s2_t.pop((mc, 2 * q + 1))[:])
        s4_t[(mc, q)] = s4

    def emit_Zb(mc, q):
        nc.tensor.matmul(zb_ps[mc][:], lhsT=ones_zb[:],
                         rhs=s4_t.pop((mc, q))[:],
                         start=(q == 0), stop=(q == NT // 4 - 1))

    def drain_z(mc, slot):
        # emit any s2/s4/Zb whose inputs landed >= 2 E-slots ago
        st = zstate[mc]
        while st[0] < NT // 2 and 2 * st[0] + 3 <= slot:
            emit_s2(mc, st[0]); st[0] += 1
        while st[1] < NT // 4 and 4 * st[1] + 6 <= slot:
            emit_s4(mc, st[1]); st[1] += 1
        while st[2] < NT // 4 and 4 * st[2] + 8 <= slot:
            emit_Zb(mc, st[2]); st[2] += 1

    def finish_z(mc):
        st = zstate[mc]
        while st[0] < NT // 2:
            emit_s2(mc, st[0]); st[0] += 1
        while st[1] < NT // 4:
            emit_s4(mc, st[1]); st[1] += 1
        while st[2] < NT // 4:
            emit_Zb(mc, st[2]); st[2] += 1

    # ---- c_attn transpose + braid of chunk-0 E/exp/Zb with c2/R ----
    catT_ps = br2.tile([DK, DK], F16, tag='catp')
    nc.tensor.transpose(catT_ps[:], cat16_sb[:], ident16[:])
    nc.vector.tensor_copy(catT_sb[:], catT_ps[:])

    start_chunk(0)
    for nt in range(NT):
        emit_E(0, nt)
        drain_z(0, nt)
        if nt < MCH:             # c2 = gamma_c*(c_attn@p)[:, msl] + pc
            sl = slice(nt * FREE, (nt + 1) * FREE)
            w_ps = br1.tile([P, FREE], F32, tag='w')
            nc.tensor.matmul(w_ps[0:DK, :], lhsT=catT_sb[:],
                             rhs=p_sb_f[:, sl], start=True, stop=True)
            nc.vector.scalar_tensor_tensor(
                out=c2_sb[:, sl], in0=w_ps[0:DK, :], scalar=gc_sb[:],
                in1=p_sb_f[:, sl], op0=ALU.mult, op1=ALU.add)
        elif nt - MCH < (MCH - 1) * CCH:   # R = Wu@c2 + bu + x[:, m]
            mc, cc = divmod(nt - MCH, CCH)   # mc 0..2; R(3) braids in chunk 3
            msl = slice(mc * FREE, (mc + 1) * FREE)
            w_ps = br1.tile([P, FREE], F32, tag='w')
            nc.tensor.matmul(w_ps[:], lhsT=wu_sb[:, cc * P:(cc + 1) * P],
                             rhs=c2_sb[:, msl], start=True, stop=True)
            cob_sb = sb2.tile([P, FREE], F32, tag='cob')
            nc.scalar.activation(cob_sb[:], w_ps[:], ACTF.Identity,
                                 bias=bu_sb[:, cc:cc + 1])
            nc.vector.tensor_add(r_sb[:, cc, msl], in0=cob_sb[:],
                                 in1=x_sb[:, cc, msl])
    finish_z(0)

    sb2_cm.__exit__(None, None, None)
    br2_cm.__exit__(None, None, None)
    br1_cm.__exit__(None, None, None)

    up_cm = tc.tile_pool(name='upool', bufs=4, space='PSUM')
    upool = up_cm.__enter__()

    # ---- main loop: normalize -> U fp8 DR, braided with next chunk's E ----
    out_r = out_d.rearrange("(kc p) m -> p kc m", p=P)
    for mc in range(MCH):
        msl = slice(mc * FREE, (mc + 1) * FREE)
        nxt = mc + 1
        if nxt < MCH:
            start_chunk(nxt)
        zrec = ssb.tile([P, FREE], F32, tag='zrec')
        nc.vector.reciprocal_approx_fast(out=zrec[:], in_=zb_ps[mc][:])
        u_ps = [upool.tile([P, FREE], F32, tag='u', name=f'u{mc}_{i}')
                for i in range(CCH)]
        r3_cobs = []
        for i in range(NT // 2):
            if nxt < MCH and i == 0:
                # cover the Zb->zrec->normalize chain at the chunk boundary
                emit_E(nxt, 0)
                emit_E(nxt, 1)
            elif nxt == MCH and i < CCH:
                # last chunk: R(3)'s matmul+bias covers the same chain
                msl3 = slice((MCH - 1) * FREE, MCH * FREE)
                w_ps = epool.tile([P, FREE], F32, tag='et')
                nc.tensor.matmul(w_ps[:],
                                 lhsT=wu_sb[:, i * P:(i + 1) * P],
                                 rhs=c2_sb[:, msl3], start=True, stop=True)
                cob_sb = otp.tile([P, FREE], F32, tag='o')
                nc.scalar.activation(cob_sb[:], w_ps[:], ACTF.Identity,
                                     bias=bu_sb[:, i:i + 1])
                r3_cobs.append(cob_sb)
            p8_t = p8pool.tile([P, 2, FREE], F8, tag='p8')
            for half in range(2):
                nc.vector.tensor_tensor(
                    p8_t[:, half, :], p_bufs[mc][:, 2 * i + half, :],
                    zrec[:], ALU.mult)
            for cc in range(CCH):
                nc.tensor.matmul(
                    u_ps[cc][:],
                    lhsT=vT8_sb[:, 2 * i:2 * i + 2, cc * P:(cc + 1) * P],
                    rhs=p8_t[:],
                    start=(i == 0), stop=(i == NT // 2 - 1),
                    perf_mode=DR)
            if nxt < MCH:
                if i > 0:
                    emit_E(nxt, 2 * i)
                    emit_E(nxt, 2 * i + 1)
                drain_z(nxt, 2 * i + 1)
            elif 2 <= i < 2 + CCH:   # deferred DVE adds for R(3)
                cc3 = i - 2
                msl3 = slice((MCH - 1) * FREE, MCH * FREE)
                nc.vector.tensor_add(r_sb[:, cc3, msl3],
                                     in0=r3_cobs[cc3][:],
                                     in1=x_sb[:, cc3, msl3])
        if nxt < MCH:
            finish_z(nxt)
        # combine: out = (gamma_s/(cp*cv)) * U + R, then store
        for cc in range(CCH):
            o_sb = otp.tile([P, FREE], F32, tag='o')
            nc.vector.scalar_tensor_tensor(
                out=o_sb[:], in0=u_ps[cc][:], scalar=gsp_sb[:],
                in1=r_sb[:, cc, msl], op0=ALU.mult, op1=ALU.add)
            nc.sync.dma_start(out_r[:, cc, msl], o_sb[:])

    up_cm.__exit__(None, None, None)
    s4_cm.__exit__(None, None, None)
    s2_cm.__exit__(None, None, None)
    ot_cm.__exit__(None, None, None)
    ss_cm.__exit__(None, None, None)
    p8_cm.__exit__(None, None, None)
    pp_cm.__exit__(None, None, None)
    zb_cm.__exit__(None, None, None)
    ep_cm.__exit__(None, None, None)
    xp16_cm.__exit__(None, None, None)
    const_cm.__exit__(None, None, None)


_CACHE = {}


def _get_compiled():
    if 'nc' in _CACHE:
        return _CACHE['nc']
    nc = bacc.Bacc("TRN2", num_devices=NCORES)
    io = {
        'x': nc.dram_tensor('x', [P, 8, KC, FREE], F16,
                            kind='ExternalInput').ap(),
        'x8': nc.dram_tensor('x8', [P, 8, KC, FREE], F8,
                             kind='ExternalInput').ap(),
        'wv8': nc.dram_tensor('wv8', [C, C], F8, kind='ExternalInput').ap(),
        'bvr': nc.dram_tensor('bvr', [1, C], F32, kind='ExternalInput').ap(),
        'wuT': nc.dram_tensor('wuT', [DK, C], F16, kind='ExternalInput').ap(),
        'consts': nc.dram_tensor('consts', [P, PKB], mybir.dt.uint8,
                                 kind='ExternalInput').ap(),
        'out': nc.dram_tensor('out', [C, M], F32, kind='ExternalOutput').ap(),
    }
    with tile.TileContext(nc) as tc:
        _build_program(tc, io)
    nc.compile()
    _CACHE['nc'] = nc
    return nc


def make_in_maps(x, Wq, bq, Wk, bk, Wv, bv, gamma_s, Wd, bd, Wu, bu, gamma_c):
    """Build the 8 per-core input dicts from the full problem inputs."""
    f32 = lambda a: np.ascontiguousarray(np.asarray(a, dtype=np.float32))
    f16 = lambda a: np.ascontiguousarray(np.asarray(a, dtype=np.float32)
                                         .astype(np.float16))
    fp8 = lambda a: np.ascontiguousarray(np.asarray(a, dtype=np.float32)
                                         .astype(ml_dtypes.float8_e4m3))
    x = f32(x).reshape(B, C, N)

    def w_chunked(wT16):  # [C, DK] f16 -> [128, KC*DK] per-partition bytes
        return np.ascontiguousarray(
            wT16.reshape(KC, P, DK).transpose(1, 0, 2).reshape(P, KC * DK))

    img = np.zeros((P, PKB), np.uint8)
    img[:, OFF_WQ:OFF_WQ + 512] = w_chunked(f16(np.asarray(Wq).T)).view(np.uint8)
    img[:, OFF_WD:OFF_WD + 512] = w_chunked(f16(np.asarray(Wd).T)).view(np.uint8)
    img[:, OFF_WK:OFF_WK + 512] = w_chunked(f16(np.asarray(Wk).T)).view(np.uint8)
    img[0:DK, OFF_BQ:OFF_BQ + 4] = f32(bq)[:, None].view(np.uint8)
    img[0:DK, OFF_BK:OFF_BK + 4] = f32(bk)[:, None].view(np.uint8)
    img[0:DK, OFF_BD:OFF_BD + 4] = f32(bd)[:, None].view(np.uint8)
    img[:, OFF_BU:OFF_BU + 16] = np.ascontiguousarray(
        f32(bu).reshape(CCH, P).T).view(np.uint8)
    img[0:DK, OFF_GC:OFF_GC + 4] = np.broadcast_to(
        f32(gamma_c)[:, None], (DK, 1)).copy().view(np.uint8)
    img[:, OFF_GSP:OFF_GSP + 4] = np.broadcast_to(
        f32(gamma_s)[:, None] / (CP * CV), (P, 1)).copy().view(np.uint8)

    shared = {
        'wv8': fp8(CV * np.asarray(Wv).T),
        'bvr': (CV * f32(bv))[None, :],
        'wuT': f16(np.asarray(Wu).T),
        'consts': img,
    }
    in_maps = []
    for core in range(NCORES):
        b, h = divmod(core, 2)
        # rotate columns so this core's m-half comes first (n-order of the
        # attention reduction is irrelevant; m-order stays canonical)
        xr = np.concatenate([x[b][:, h * M:(h + 1) * M],
                             x[b][:, (1 - h) * M:(2 - h) * M]], axis=1)
        xc = np.ascontiguousarray(
            xr.reshape(KC, P, 8, FREE).transpose(1, 2, 0, 3))
        in_maps.append({
            'x': f16(xc),
            'x8': fp8(xc),
            **shared,
        })
    return in_maps


def assemble_out(results):
    """Stitch the 8 per-core [C, M] outputs back to [B, C, W, H]."""
    full = np.empty((B, C, N), np.float32)
    for core, res in enumerate(results):
        b, h = divmod(core, 2)
        full[b][:, h * M:(h + 1) * M] = res['out']
    return full.reshape(B, C, WIDTH, HEIGHT)


def kernel(**inputs):
    nc = _get_compiled()
    in_maps = make_in_maps(**inputs)
    res = bass_utils.run_bass_kernel_spmd(nc, in_maps, core_ids=list(range(NCORES)))
    return assemble_out(res.results)
